# revision 16
# baseline (speedup 1.0000x reference)
"""Trainium2 Bass kernel for nn_CrystalGraphNetwork (4-layer GNN message
passing, N=50000 nodes, E=800000 edges, H=128, G=256 graphs) on 8 NeuronCores.

Strategy (SPMD, one program, per-core data):
  - Nodes sharded contiguously (6250/core); edges sharded by receiver
    (receivers are sorted, so edge ranges are contiguous).
  - Message MLP layer 2 (mw2/mb2) is folded into the update MLP host-side:
    agg @ (mw2 @ uw1_bot) + deg * (mb2 @ uw1_bot); the device only computes
    m1 = silu(ys[sender] + yr[receiver] + ef @ W1c) per edge, where
    ys = h @ mw1[:128], yr = h @ mw1[128:256] + mb1 are per-node tables.
  - ys table is AllGather'ed across cores each layer (bf16); yr is local.
  - Per-edge gathers via the dma_gather custom instruction (bf16 rows,
    int16 indices -> sender table split at row 32768 into lo/hi halves;
    edges grouped per 128-node receiver window and sorted lo/hi within it).
  - Segment-sum over sorted receivers as PE matmuls with one-hot S tiles
    generated on DVE (is_equal against an iota constant).
  - Graph mean-pool via the same one-hot trick + AllReduce; final MLP
    computed redundantly on every core; core 0's output is returned.
"""

import numpy as np
import ml_dtypes

N, E, G = 50000, 800000, 256
H, NB = 128, 8
NLAYERS = 4
CUTOFF = 4.0
EDGE_F = NB + 9
NCORES = 8
NC = N // NCORES
NCPAD = ((NC + 127) // 128) * 128
NWIN = NCPAD // 128
LO = 32768
GB_TILES = 16                  # gather batch: 16 tiles = 2048 indices
GB = GB_TILES * 128

BF16 = ml_dtypes.bfloat16


# ---------------------------------------------------------------------------
# Host-side preprocessing
# ---------------------------------------------------------------------------

def edge_features_np(edges):
    x, y, z = edges[:, 0], edges[:, 1], edges[:, 2]
    dist = np.sqrt(x * x + y * y + z * z)
    scaled = np.clip(dist / CUTOFF, 0.0, 1.0)
    freqs = np.arange(1, NB + 1, dtype=np.float32) * np.float32(np.pi)
    arg = scaled[:, None] * freqs
    radial = np.sin(arg) / arg
    eps = 1e-7
    u = edges / (dist + eps)[:, None]
    ux, uy, uz = u[:, 0], u[:, 1], u[:, 2]
    c0, c1 = 0.28209479177387814, 0.4886025119029199
    c2a, c2b, c2c = 1.0925484305920792, 0.31539156525252005, 0.5462742152960396
    ang = np.stack([c0 * np.ones_like(ux), c1 * uy, c1 * uz, c1 * ux,
                    c2a * ux * uy, c2a * uy * uz, c2b * (3.0 * uz * uz - 1.0),
                    c2a * ux * uz, c2c * (ux * ux - uy * uy)], axis=-1)
    return np.concatenate([radial, ang], axis=-1).astype(np.float32)


def fold_weights(params):
    out = []
    for p in params["layers"]:
        mw1 = np.asarray(p["mw1"], np.float32)
        uw1 = np.asarray(p["uw1"], np.float32)
        mw2 = np.asarray(p["mw2"], np.float32)
        out.append(dict(
            W1a=mw1[0:H], W1b=mw1[H:2 * H], W1c=mw1[2 * H:2 * H + EDGE_F],
            b1=np.asarray(p["mb1"], np.float32),
            uw1t=uw1[0:H], uw1b_f=mw2 @ uw1[H:2 * H],
            c2=np.asarray(p["mb2"], np.float32) @ uw1[H:2 * H],
            ub1=np.asarray(p["ub1"], np.float32),
            uw2=np.asarray(p["uw2"], np.float32),
            ub2=np.asarray(p["ub2"], np.float32)))
    return out


def build_structure(senders, receivers):
    """SPMD-uniform tile structure. See prep.py for the validated model."""
    senders = np.asarray(senders)
    receivers = np.asarray(receivers)
    e_lo = np.searchsorted(receivers, np.arange(NCORES) * NC, side="left")
    e_hi = np.searchsorted(receivers, (np.arange(NCORES) + 1) * NC, side="left")

    lo_ids = [[None] * NWIN for _ in range(NCORES)]
    hi_ids = [[None] * NWIN for _ in range(NCORES)]
    for c in range(NCORES):
        e0, e1 = e_lo[c], e_hi[c]
        r = receivers[e0:e1] - c * NC
        wb = np.searchsorted(r, np.arange(NWIN + 1) * 128, side="left")
        for w in range(NWIN):
            ids = np.arange(e0 + wb[w], e0 + wb[w + 1])
            m = senders[ids] < LO
            lo_ids[c][w] = ids[m]
            hi_ids[c][w] = ids[~m]

    LC = np.zeros(NWIN, np.int64)
    HC = np.zeros(NWIN, np.int64)
    for w in range(NWIN):
        LC[w] = max((len(lo_ids[c][w]) + 127) // 128 for c in range(NCORES))
        HC[w] = max((len(hi_ids[c][w]) + 127) // 128 for c in range(NCORES))
        if LC[w] + HC[w] == 0:
            LC[w] = 1  # guarantee >=1 tile so every window psum is written

    SLO, SHI = int(LC.sum()) * 128, int(HC.sum()) * 128
    S = SLO + SHI
    st = dict(LC=LC, HC=HC, SLO=SLO, SHI=SHI, S=S, cores=[])

    # per-tile metadata, identical across cores (process order)
    tiles = []           # (window, kind, stream_tile_idx)
    plo = phi = 0
    for w in range(NWIN):
        for t in range(int(LC[w])):
            tiles.append((w, 0, plo)); plo += 1
        for t in range(int(HC[w])):
            tiles.append((w, 1, phi)); phi += 1
    st["tiles"] = tiles

    for c in range(NCORES):
        ys_lo = np.zeros(SLO, np.int16)
        ys_hi = np.zeros(SHI, np.int16)
        yr = np.zeros(S, np.int16)
        rrec = np.full(S, 1000.0, np.float32)
        eperm = np.full(S, -1, np.int64)
        plo = phi = pp = 0
        for w in range(NWIN):
            for kind in (0, 1):
                ids = lo_ids[c][w] if kind == 0 else hi_ids[c][w]
                nt = int(LC[w] if kind == 0 else HC[w])
                ns, n = nt * 128, len(ids)
                seg = np.zeros(ns, np.int16)
                seg[:n] = (senders[ids] - (LO if kind else 0)).astype(np.int16)
                if kind == 0:
                    ys_lo[plo:plo + ns] = seg; plo += ns
                else:
                    ys_hi[phi:phi + ns] = seg; phi += ns
                yseg = np.zeros(ns, np.int16)
                yseg[:n] = (receivers[ids] - c * NC).astype(np.int16)
                yr[pp:pp + ns] = yseg
                rseg = np.full(ns, 1000.0, np.float32)
                rseg[:n] = (receivers[ids] - c * NC - w * 128).astype(np.float32)
                rrec[pp:pp + ns] = rseg
                es = np.full(ns, -1, np.int64)
                es[:n] = ids
                eperm[pp:pp + ns] = es
                pp += ns
        st["cores"].append(dict(ys_lo_idx=ys_lo, ys_hi_idx=ys_hi, yr_idx=yr,
                                rel_recv=rrec, eperm=eperm))
    return st


def wrap_idx(idx):
    """[S] int16 -> [128, S/16] wrapped (i at [i%16, i//16]) and replicated."""
    s = len(idx)
    return np.tile(idx.reshape(s // 16, 16).T, (8, 1)).copy()


def col_wrap(a, rows=128):
    """[k*rows] -> [rows, k]: element (p, j) = a[j*rows + p]."""
    return np.asarray(a).reshape(-1, rows).T.copy()


def make_quads(st):
    """Group process-order tiles into matmul quads: runs of <=4 tiles with the
    same (window, kind) that do not cross a gather-batch (GB_TILES) boundary
    in either their ys stream or the process-order (yr) stream."""
    quads = []
    cur = []
    for t, (w, kind, sidx) in enumerate(st["tiles"]):
        if cur:
            w0, k0, s0, t0 = cur[0]
            ok = (w == w0 and kind == k0 and len(cur) < 4
                  and sidx == s0 + len(cur)
                  and sidx // GB_TILES == s0 // GB_TILES
                  and t // GB_TILES == t0 // GB_TILES)
            if not ok:
                quads.append(cur)
                cur = []
        cur.append((w, kind, sidx, t))
    if cur:
        quads.append(cur)
    return quads


# ---------------------------------------------------------------------------
# Bass program
# ---------------------------------------------------------------------------

def build_bass(st):
    import concourse.bacc as bacc
    import concourse.mybir as mybir
    import concourse.tile as tile

    F32 = mybir.dt.float32
    BF = mybir.dt.bfloat16
    I16 = mybir.dt.int16
    AF = mybir.ActivationFunctionType
    Alu = mybir.AluOpType

    SLO, SHI, S = st["SLO"], st["SHI"], st["S"]
    T = S // 128
    LC, HC = st["LC"], st["HC"]
    quads = make_quads(st)

    nc = bacc.Bacc("TRN2", num_devices=NCORES, num_swdge_queues=1)

    # ---- inputs ----
    nodes_fm_loc = nc.dram_tensor("nodes_fm_loc", [H, NCPAD], F32, kind="ExternalInput")
    efT_d = nc.dram_tensor("efT", [EDGE_F, S], BF, kind="ExternalInput")
    rel_recv_d = nc.dram_tensor("rel_recv", [128, T], F32, kind="ExternalInput")
    rel_gid_d = nc.dram_tensor("rel_gid", [128, 2 * NWIN], F32, kind="ExternalInput")
    deg_d = nc.dram_tensor("deg", [1, NCPAD], F32, kind="ExternalInput")
    yslo_idx_d = nc.dram_tensor("yslo_idx", [128, SLO // 16], I16, kind="ExternalInput")
    yshi_idx_d = nc.dram_tensor("yshi_idx", [128, SHI // 16], I16, kind="ExternalInput")
    yr_idx_d = nc.dram_tensor("yr_idx", [128, S // 16], I16, kind="ExternalInput")
    iota_d = nc.dram_tensor("iota", [128, 128], F32, kind="ExternalInput")
    ident_d = nc.dram_tensor("ident_bf", [128, 128], BF, kind="ExternalInput")
    identf_d = nc.dram_tensor("ident_f32", [128, 128], F32, kind="ExternalInput")
    ones_d = nc.dram_tensor("ones", [128, 1], F32, kind="ExternalInput")

    wnames_f32 = ["W1a", "W1b", "b1bc", "uw1t", "uw1b", "uw2"]      # [4,128,128]
    wcol_f32 = ["ub1", "ub2"]                                       # [4,128,1]
    w_d = {n: nc.dram_tensor(n, [NLAYERS, H, H], F32, kind="ExternalInput")
           for n in wnames_f32}
    w_d.update({n: nc.dram_tensor(n, [NLAYERS, H, 1], F32, kind="ExternalInput")
                for n in wcol_f32})
    w_d["c2"] = nc.dram_tensor("c2", [NLAYERS, 1, H], F32, kind="ExternalInput")
    w_d["W1c"] = nc.dram_tensor("W1c", [NLAYERS, EDGE_F, H], BF, kind="ExternalInput")
    ow1_d = nc.dram_tensor("ow1", [H, H], F32, kind="ExternalInput")
    ob1_d = nc.dram_tensor("ob1", [H, 1], F32, kind="ExternalInput")
    ow2_d = nc.dram_tensor("ow2", [H, 1], F32, kind="ExternalInput")
    ob2_d = nc.dram_tensor("ob2", [1, 1], F32, kind="ExternalInput")

    out_d = nc.dram_tensor("out", [1, G], F32, kind="ExternalOutput")

    # ---- internal DRAM ----
    ys_table = nc.dram_tensor("ys_table", [N, H], BF)
    ys_local = nc.dram_tensor("ys_local", [NC, H], BF)
    yr_local = nc.dram_tensor("yr_local", [NC, H], BF)
    gs_loc = nc.dram_tensor("gs_loc", [G, 129], F32)
    gs_glob = nc.dram_tensor("gs_glob", [G, 129], F32)

    with tile.TileContext(nc) as tc:
        with (
            tc.tile_pool(name="const", bufs=1) as constp,
            tc.tile_pool(name="resident", bufs=1) as resp,
            tc.tile_pool(name="wts", bufs=2) as wtsp,
        ):
            # constants
            iota_sb = constp.tile([128, 128], F32)
            ident_sb = constp.tile([128, 128], BF)
            identf_sb = constp.tile([128, 128], F32)
            ones_sb = constp.tile([128, 1], F32)
            rel_recv_sb = constp.tile([128, T], F32)
            rel_gid_sb = constp.tile([128, 2 * NWIN], F32)
            nc.sync.dma_start(iota_sb[:], iota_d[:])
            nc.sync.dma_start(ident_sb[:], ident_d[:])
            nc.sync.dma_start(identf_sb[:], identf_d[:])
            nc.sync.dma_start(ones_sb[:], ones_d[:])
            nc.sync.dma_start(rel_recv_sb[:], rel_recv_d[:])
            nc.sync.dma_start(rel_gid_sb[:], rel_gid_d[:])

            # persistent node state (feature-major), ping-pong across layers
            h_a = resp.tile([128, NCPAD], F32, tag="h_a")
            h_b = resp.tile([128, NCPAD], F32, tag="h_b")
            nc.sync.dma_start(h_a[:], nodes_fm_loc[:])

            with (
                tc.tile_pool(name="nodework", bufs=3) as nodep,
                tc.tile_pool(name="gath", bufs=2) as gathp,
                tc.tile_pool(name="edgework", bufs=3) as edgep,
                tc.tile_pool(name="aggp", bufs=1) as aggp_pool,
                tc.tile_pool(name="psum", bufs=2, space="PSUM") as psump,
            ):
                for li in range(NLAYERS):
                    h_cur = h_a if li % 2 == 0 else h_b
                    h_nxt = h_b if li % 2 == 0 else h_a

                    # --- layer weights ---
                    wt = {}
                    for nm in wnames_f32:
                        wt[nm] = wtsp.tile([H, H], F32, tag=nm, name="wt_" + nm)
                        nc.sync.dma_start(wt[nm][:], w_d[nm][li])
                    for nm in wcol_f32:
                        wt[nm] = wtsp.tile([H, 1], F32, tag=nm, name="wt_" + nm)
                        nc.sync.dma_start(wt[nm][:], w_d[nm][li])
                    wt["c2"] = wtsp.tile([1, H], F32, tag="c2", name="wt_c2")
                    nc.sync.dma_start(wt["c2"][:], w_d["c2"][li])
                    wt["W1c"] = wtsp.tile([EDGE_F, H], BF, tag="W1c", name="wt_W1c")
                    nc.sync.dma_start(wt["W1c"][:], w_d["W1c"][li])

                    # --- phase A: ys / yr tables ---
                    if True:
                        for ch in range(NWIN):
                            cw = min(128, NC - ch * 128)
                            ps = psump.tile([128, 512], F32, tag="node")
                            nc.tensor.matmul(ps[:cw, :H],
                                             h_cur[:, ch * 128:ch * 128 + cw],
                                             wt["W1a"][:])
                            sb = nodep.tile([128, H], BF, tag="nout")
                            nc.scalar.activation(sb[:cw, :], ps[:cw, :H], AF.Copy)
                            nc.sync.dma_start(ys_local[ch * 128:ch * 128 + cw], sb[:cw, :])
                        nc.gpsimd.collective_compute(
                            "AllGather", mybir.AluOpType.bypass,
                            replica_groups=[list(range(NCORES))],
                            ins=[ys_local[:].opt()], outs=[ys_table[:].opt()])

                    for ch in range(NWIN):
                        cw = min(128, NC - ch * 128)
                        ps = psump.tile([128, 512], F32, tag="node")
                        nc.tensor.matmul(ps[:cw, :H], h_cur[:, ch * 128:ch * 128 + cw],
                                         wt["W1b"][:])
                        sb = nodep.tile([128, H], BF, tag="nout")
                        nc.vector.tensor_tensor(sb[:cw, :], ps[:cw, :H],
                                                wt["b1bc"][:cw, :], Alu.add)
                        nc.sync.dma_start(yr_local[ch * 128:ch * 128 + cw], sb[:cw, :])

                    # --- phase B: edge loop ---
                    agg_fm = aggp_pool.tile([128, NCPAD], F32, tag="agg")
                    if NCPAD > NC:
                        nc.vector.memset(agg_fm[:, NC:], 0.0)

                    gath_bufs = {}

                    def gather_call(kind, call):
                        key = (kind, call)
                        if key in gath_bufs:
                            return gath_bufs[key]
                        tot = {0: SLO, 1: SHI, 2: S}[kind]
                        n = min(GB, tot - call * GB)
                        idx_dram = {0: yslo_idx_d, 1: yshi_idx_d, 2: yr_idx_d}[kind]
                        tag = {0: "glo", 1: "ghi", 2: "gyr"}[kind]
                        it = edgep.tile([128, GB // 16], I16, tag=tag + "i")
                        nc.sync.dma_start(it[:, :n // 16],
                                          idx_dram[:, call * (GB // 16):call * (GB // 16) + n // 16])
                        buf = gathp.tile([128, GB_TILES, 128], BF, tag=tag)
                        if kind == 0:
                            src = ys_table[0:LO]
                        elif kind == 1:
                            src = ys_table[LO:N]
                        else:
                            src = yr_local[:]
                        nc.gpsimd.dma_gather(buf[:, :n // 128, :], src, it[:, :n // 16],
                                             n, n, H, queue_num=0,
                                             single_packet=False)
                        gath_bufs[key] = buf
                        return buf

                    def efbuf(call):
                        key = ("ef", call)
                        if key in gath_bufs:
                            return gath_bufs[key]
                        n = min(GB, S - call * GB)
                        buf = edgep.tile([EDGE_F, GB], BF, tag="ef", bufs=2)
                        nc.sync.dma_start(buf[:, :n], efT_d[:, call * GB:call * GB + n])
                        gath_bufs[key] = buf
                        return buf

                    # group quads per window (tiles are in window-major order)
                    win_quads = [[] for _ in range(NWIN)]
                    for q in quads:
                        win_quads[q[0][0]].append(q)

                    for w in range(NWIN):
                        cw = min(128, NC - w * 128)
                        wps = psump.tile([128, 128], F32, tag="aggps")
                        nchunks = int(LC[w] + HC[w])
                        ci = 0
                        for q in win_quads[w]:
                            _, kind, s0, t0 = q[0]
                            nq = len(q)
                            ysb = gather_call(kind, s0 // GB_TILES)
                            yso = s0 % GB_TILES
                            yrb = gather_call(2, t0 // GB_TILES)
                            yro = t0 % GB_TILES
                            efb = efbuf(t0 // GB_TILES)
                            efo = (t0 % GB_TILES) * 128

                            m1ps = psump.tile([128, 512], F32, tag="m1pre")
                            nc.tensor.matmul(
                                m1ps[:, :nq * 128], ident_sb[:],
                                ysb[:, yso:yso + nq, :].rearrange("p a b -> p (a b)"),
                                start=True, stop=False)
                            for j in range(nq):
                                nc.tensor.matmul(
                                    m1ps[:, j * 128:(j + 1) * 128],
                                    efb[:, efo + j * 128:efo + (j + 1) * 128],
                                    wt["W1c"][:], start=False, stop=False)
                            nc.tensor.matmul(
                                m1ps[:, :nq * 128], ident_sb[:],
                                yrb[:, yro:yro + nq, :].rearrange("p a b -> p (a b)"),
                                start=False, stop=True)
                            m1 = edgep.tile([128, 512], BF, tag="m1")
                            nc.scalar.activation(m1[:, :nq * 128], m1ps[:, :nq * 128],
                                                 AF.Silu)
                            for j in range(nq):
                                t = t0 + j
                                s_t = edgep.tile([128, 128], BF, tag="s")
                                nc.vector.tensor_scalar(
                                    s_t[:], iota_sb[:], rel_recv_sb[:, t:t + 1],
                                    None, Alu.is_equal)
                                nc.tensor.matmul(
                                    wps[:], m1[:, j * 128:(j + 1) * 128], s_t[:],
                                    start=(ci == 0), stop=(ci == nchunks - 1))
                                ci += 1
                        nc.scalar.activation(agg_fm[:, w * 128:w * 128 + cw],
                                             wps[:, :cw], AF.Copy)

                    # --- phase C: update MLP ---
                    for s in range(0, NCPAD, 512):
                        wd = min(512, NCPAD - s)
                        ps = psump.tile([128, 512], F32, tag="node")
                        nc.tensor.matmul(ps[:, :wd], wt["uw1t"][:],
                                         h_cur[:, s:s + wd], start=True, stop=False)
                        nc.tensor.matmul(ps[:, :wd], wt["uw1b"][:],
                                         agg_fm[:, s:s + wd], start=False, stop=False)
                        degc = nodep.tile([1, 512], F32, tag="degc")
                        nc.sync.dma_start(degc[:, :wd], deg_d[:, s:s + wd])
                        nc.tensor.matmul(ps[:, :wd], wt["c2"][:],
                                         degc[:, :wd], start=False, stop=True)
                        u1 = nodep.tile([128, 512], F32, tag="u1")
                        nc.scalar.activation(u1[:, :wd], ps[:, :wd], AF.Silu,
                                             bias=wt["ub1"][:])
                        ps2 = psump.tile([128, 512], F32, tag="node")
                        nc.tensor.matmul(ps2[:, :wd], wt["uw2"][:], u1[:, :wd])
                        nc.scalar.activation(h_nxt[:, s:s + wd], ps2[:, :wd], AF.Identity,
                                             bias=wt["ub2"][:])

            # ---- graph pooling + readout ----
            h_fin = h_a if NLAYERS % 2 == 0 else h_b
            with (
                tc.tile_pool(name="gwork", bufs=3) as gwp,
                tc.tile_pool(name="gpsum", bufs=1, space="PSUM") as gps,
                tc.tile_pool(name="gpsum2", bufs=2, space="PSUM") as gps2,
            ):
                pg = [gps.tile([128, 129], F32, tag=f"gsum{g}", name=f"gsum{g}") for g in range(2)]
                for w in range(NWIN):
                    cw = min(128, NC - w * 128)
                    pt = gps2.tile([128, 128], F32, tag="tr")
                    nc.tensor.transpose(pt[:cw, :], h_fin[:, w * 128:w * 128 + cw],
                                        identf_sb[:])
                    hnm = gwp.tile([128, 129], F32, tag="hnm")
                    nc.scalar.activation(hnm[:cw, :128], pt[:cw, :], AF.Copy)
                    nc.vector.memset(hnm[:cw, 128:129], 1.0)
                    for g in range(2):
                        sg = gwp.tile([128, 128], F32, tag="sg")
                        nc.vector.tensor_scalar(
                            sg[:cw, :], iota_sb[:cw, :],
                            rel_gid_sb[:cw, g * NWIN + w:g * NWIN + w + 1], None,
                            Alu.is_equal)
                        nc.tensor.matmul(pg[g][:], sg[:cw, :], hnm[:cw, :],
                                         start=(w == 0), stop=(w == NWIN - 1))
                for g in range(2):
                    gsb = gwp.tile([128, 129], F32, tag="gsb")
                    nc.scalar.activation(gsb[:], pg[g][:], AF.Copy)
                    nc.sync.dma_start(gs_loc[g * 128:(g + 1) * 128], gsb[:])
                nc.gpsimd.collective_compute(
                    "AllReduce", mybir.AluOpType.add,
                    replica_groups=[list(range(NCORES))],
                    ins=[gs_loc[:].opt()], outs=[gs_glob[:].opt()])

                gf_fm = gwp.tile([128, G], F32, tag="gffm")
                for g in range(2):
                    gsb = gwp.tile([128, 129], F32, tag="gsb2")
                    nc.sync.dma_start(gsb[:], gs_glob[g * 128:(g + 1) * 128])
                    cnt = gwp.tile([128, 1], F32, tag="cnt")
                    nc.vector.tensor_scalar_max(cnt[:], gsb[:, 128:129], 1.0)
                    inv = gwp.tile([128, 1], F32, tag="inv")
                    nc.vector.reciprocal(inv[:], cnt[:])
                    gfc = gwp.tile([128, 128], F32, tag="gfc")
                    nc.vector.tensor_scalar_mul(gfc[:], gsb[:, 0:128], inv[:])
                    pt = gps2.tile([128, 128], F32, tag="tr")
                    nc.tensor.transpose(pt[:], gfc[:], identf_sb[:])
                    nc.scalar.activation(gf_fm[:, g * 128:(g + 1) * 128], pt[:],
                                         AF.Copy)

                ow1_sb = gwp.tile([H, H], F32, tag="ow1")
                ob1_sb = gwp.tile([H, 1], F32, tag="ob1")
                ow2_sb = gwp.tile([H, 1], F32, tag="ow2")
                ob2_sb = gwp.tile([1, 1], F32, tag="ob2")
                nc.sync.dma_start(ow1_sb[:], ow1_d[:])
                nc.sync.dma_start(ob1_sb[:], ob1_d[:])
                nc.sync.dma_start(ow2_sb[:], ow2_d[:])
                nc.sync.dma_start(ob2_sb[:], ob2_d[:])
                po1 = gps2.tile([128, G], F32, tag="o1")
                nc.tensor.matmul(po1[:], ow1_sb[:], gf_fm[:])
                o1 = gwp.tile([128, G], F32, tag="o1sb")
                nc.scalar.activation(o1[:], po1[:], AF.Silu, bias=ob1_sb[:])
                po2 = gps2.tile([1, G], F32, tag="o2")
                nc.tensor.matmul(po2[:], ow2_sb[:], o1[:])
                osb = gwp.tile([1, G], F32, tag="osb")
                nc.scalar.activation(osb[:], po2[:], AF.Identity, bias=ob2_sb[:])
                nc.sync.dma_start(out_d[:], osb[:])

    nc.compile()
    return nc


# ---------------------------------------------------------------------------
# Entry point
# ---------------------------------------------------------------------------

def make_in_maps(nodes, edges, senders, receivers, node_graph_ids, params, st):
    layers = fold_weights(params)[:NLAYERS]
    ef = edge_features_np(np.asarray(edges, np.float32))
    pad_ef = edge_features_np(np.array([[1.0, 0.0, 0.0]], np.float32))[0]

    nodes = np.asarray(nodes, np.float32)
    gids = np.asarray(node_graph_ids)
    common = dict(
        iota=np.tile(np.arange(128, dtype=np.float32), (128, 1)),
        ident_bf=np.eye(128, dtype=np.float32).astype(BF16),
        ident_f32=np.eye(128, dtype=np.float32),
        ones=np.ones((128, 1), np.float32),
        ow1=np.asarray(params["ow1"], np.float32),
        ob1=np.asarray(params["ob1"], np.float32).reshape(H, 1),
        ow2=np.asarray(params["ow2"], np.float32).reshape(H, 1),
        ob2=np.asarray(params["ob2"], np.float32).reshape(1, 1),
    )
    for nm, getter in [
        ("W1a", lambda l: l["W1a"]), ("W1b", lambda l: l["W1b"]),
        ("b1bc", lambda l: np.tile(l["b1"], (128, 1))),
        ("uw1t", lambda l: l["uw1t"]), ("uw1b", lambda l: l["uw1b_f"]),
        ("uw2", lambda l: l["uw2"]),
    ]:
        common[nm] = np.stack([getter(l) for l in layers]).astype(np.float32)
    common["ub1"] = np.stack([l["ub1"].reshape(H, 1) for l in layers]).astype(np.float32)
    common["ub2"] = np.stack([l["ub2"].reshape(H, 1) for l in layers]).astype(np.float32)
    common["c2"] = np.stack([l["c2"].reshape(1, H) for l in layers]).astype(np.float32)
    common["W1c"] = np.stack([l["W1c"] for l in layers]).astype(BF16)

    in_maps = []
    for c in range(NCORES):
        stc = st["cores"][c]
        ef_slot = np.empty((st["S"], EDGE_F), np.float32)
        real = stc["eperm"] >= 0
        ef_slot[real] = ef[stc["eperm"][real]]
        ef_slot[~real] = pad_ef
        deg = np.zeros(NCPAD, np.float32)
        rcv = np.asarray(receivers)
        mine = rcv[(rcv >= c * NC) & (rcv < (c + 1) * NC)] - c * NC
        np.add.at(deg, mine, 1.0)
        hloc = np.zeros((H, NCPAD), np.float32)
        hloc[:, :NC] = nodes[c * NC:(c + 1) * NC].T
        relg = np.full((128, 2 * NWIN), 1000.0, np.float32)
        gl = gids[c * NC:(c + 1) * NC].astype(np.float32)
        glp = np.full(NCPAD, 100000.0, np.float32)
        glp[:NC] = gl
        for g in range(2):
            relg[:, g * NWIN:(g + 1) * NWIN] = col_wrap(glp) - g * 128
        m = dict(common)
        m.update(
            nodes_fm_loc=hloc,
            efT=ef_slot.T.copy().astype(BF16),
            rel_recv=col_wrap(stc["rel_recv"]),
            rel_gid=relg,
            deg=deg.reshape(1, NCPAD),
            yslo_idx=wrap_idx(stc["ys_lo_idx"]),
            yshi_idx=wrap_idx(stc["ys_hi_idx"]),
            yr_idx=wrap_idx(stc["yr_idx"]),
        )
        in_maps.append(m)
    return in_maps


_cache = {}


def kernel(nodes, edges, senders, receivers, node_graph_ids, params):
    from concourse.bass_utils import run_bass_kernel_spmd

    st = build_structure(senders, receivers)
    in_maps = make_in_maps(nodes, edges, senders, receivers, node_graph_ids,
                           params, st)
    key = "nc"
    if key not in _cache:
        _cache[key] = build_bass(st)
    res = run_bass_kernel_spmd(_cache[key], in_maps, core_ids=list(range(NCORES)))
    return np.asarray(res.results[0]["out"], np.float32).reshape(G, 1)
